# revision 1
# baseline (speedup 1.0000x reference)
"""Trainium2 Bass kernel for SoftPathMDDLoss.

Math: with z_t = BETA * log_w_t, the loss reduces to
    G = sum_t exp(B p_t - z_t) = sum_t V_t,   V_t = e^{-B u_t} V_{t-1} + 1
(V_t = exp(BETA * (soft_peak_t - log_w_t)) — the running soft-drawdown factor,
an affine recurrence in linear space). V overflows f32 over the full path, so
the time axis is cut into chunks of C steps; each chunk is evaluated with
chunk-local state on device and the cross-chunk affine composition runs on the
host in float64 log-space.

Per chunk the device emits 4 scalars per path:
    sig  = sum(ln(1+r))               (ACT accum; host: Wend = e^{-BETA*sig})
    gi   = sum_t V'_t                 (chunk-local V scan + GPSIMD cumsum)
    vend = V'_{C-1}
    hst  = reverse affine scan head = 1 + sum_t W_t  (W = cumprod e^{-B u})
Host: G accumulates as  Sum_k [gi_k + Dt_k * Vstart_k],
      Vstart_{k+1} = vend_k + Wend_k * Vstart_k,  Dt_k = hst_k - 1.

Layout: paths on partitions (128 per core = B/8, data parallel over 8 cores),
time on the free axis. Scans run on DVE (tensor_tensor_scan), log1p/exp on
ACT, the V-cumsum on GPSIMD.
"""
import numpy as np

import concourse.bacc as bacc
import concourse.mybir as mybir
import concourse.tile as tile
from concourse.bass_utils import run_bass_kernel_spmd

T, B = 65536, 1024
NCORES = 8
BL = B // NCORES          # 128 paths per core (partition dim)
C = 1024                  # chunk length (time steps)
K = T // C                # chunks per core
NCB = 4                   # chunks per input DMA block / scan merge
NBLK = K // NCB           # input blocks per core
BETA = 40.0
THRESHOLD = 0.12          # -MDD_TARGET + SOFT_MARGIN
MDD_LAMBDA = 5.0

_F32 = mybir.dt.float32
_AF = mybir.ActivationFunctionType
_ALU = mybir.AluOpType

_built = None


def _patch_act_tables():
    """Put natural_log_exp_and_others first so Ln/Exp/Copy resolve to ONE
    table set (otherwise bacc alternates table loads, ~1.3us each)."""
    import concourse.hw_specs as hw_specs

    if getattr(bacc, "_act_tables_patched", False):
        return
    orig = hw_specs.get_activation_tables

    def patched(arch):
        t = orig(arch)
        pref = "natural_log_exp_and_others"
        if pref not in t:
            return t
        # keep canonical order (act_func_set_id is positional); make the
        # combined set the only provider of Exp/Copy so no table swapping
        strip = {mybir.ActivationFunctionType.Exp,
                 mybir.ActivationFunctionType.Copy}
        return {k: (v if k == pref else (v - strip)) for k, v in t.items()}

    bacc.get_activation_tables = patched
    bacc._act_tables_patched = True


def _build():
    global _built
    if _built is not None:
        return _built
    _patch_act_tables()
    nc = bacc.Bacc("TRN2", target_bir_lowering=False, debug=False)
    x = nc.dram_tensor("xt", [BL, T], _F32, kind="ExternalInput").ap()
    out_sig = nc.dram_tensor("sig", [BL, K], _F32, kind="ExternalOutput").ap()
    out_hst = nc.dram_tensor("hst", [BL, K], _F32, kind="ExternalOutput").ap()
    out_vend = nc.dram_tensor("vend", [BL, K], _F32, kind="ExternalOutput").ap()
    out_gi = nc.dram_tensor("gi", [BL, K], _F32, kind="ExternalOutput").ap()

    with tile.TileContext(nc) as tc:
        with tc.tile_pool(name="io", bufs=2) as iop, \
             tc.tile_pool(name="work", bufs=2) as wp, \
             tc.tile_pool(name="chunk", bufs=2) as cp, \
             tc.tile_pool(name="stat", bufs=1) as sp:
            ones = sp.tile([BL, NCB * C], _F32, tag="ones", name="ones")
            nc.vector.memset(ones[:], 1.0)
            st_s = sp.tile([BL, K], _F32, tag="st_s", name="st_s")
            st_h = sp.tile([BL, K], _F32, tag="st_h", name="st_h")
            st_v = sp.tile([BL, K], _F32, tag="st_v", name="st_v")
            st_g = sp.tile([BL, K], _F32, tag="st_g", name="st_g")

            for j in range(NBLK):
                bt = iop.tile([BL, NCB * C], _F32, tag="bt", name="bt")
                nc.sync.dma_start(out=bt[:],
                                  in_=x[:, j * NCB * C:(j + 1) * NCB * C])
                iab = wp.tile([BL, NCB * C], _F32, tag="iab", name="iab")
                for c in range(NCB):
                    kk = j * NCB + c
                    sl = bt[:, c * C:(c + 1) * C]
                    # in-place log1p; accum gives the chunk sum of ln(1+r)
                    nc.scalar.activation(sl, sl, _AF.Ln, bias=1.0, scale=1.0,
                                         accum_out=st_s[:, kk:kk + 1])
                    nc.scalar.activation(iab[:, c * C:(c + 1) * C], sl,
                                         _AF.Exp, bias=0.0, scale=-BETA)
                    # zero the chunk-start multiplier: the merged scans then
                    # self-reset at chunk boundaries (V=1 fwd, H=1 rev); the
                    # host restores Dt = ia_0 * H_1 from the raw input
                    nc.gpsimd.memset(iab[:, c * C:c * C + 1], 0.0)
                # one fwd and one rev scan per block (2 cyc/elem either way;
                # merging amortizes the per-instruction overhead)
                vtb = cp.tile([BL, NCB * C], _F32, tag="vtb", name="vtb")
                nc.vector.tensor_tensor_scan(vtb[:], iab[:], ones[:], 0.0,
                                             op0=_ALU.mult, op1=_ALU.add)
                htb = cp.tile([BL, NCB * C], _F32, tag="htb", name="htb")
                nc.vector.tensor_tensor_scan(htb[:, ::-1], iab[:, ::-1],
                                             ones[:], 1.0,
                                             op0=_ALU.mult, op1=_ALU.add)
                for c in range(NCB):
                    kk = j * NCB + c
                    sc = cp.tile([BL, C], _F32, tag="sc", name="sc", bufs=3)
                    nc.scalar.activation(sc[:], vtb[:, c * C:(c + 1) * C],
                                         _AF.Copy, bias=0.0, scale=1.0,
                                         accum_out=st_g[:, kk:kk + 1])
                nc.vector.tensor_copy(st_h[:, j * NCB:(j + 1) * NCB],
                                      htb[:, 1::C])
                nc.vector.tensor_copy(st_v[:, j * NCB:(j + 1) * NCB],
                                      vtb[:, C - 1::C])

            nc.sync.dma_start(out=out_sig, in_=st_s[:])
            nc.sync.dma_start(out=out_hst, in_=st_h[:])
            nc.sync.dma_start(out=out_vend, in_=st_v[:])
            nc.sync.dma_start(out=out_gi, in_=st_g[:])

    nc.compile()
    _built = nc
    return nc


def _combine(sig, hst, vend, gi, r0):
    """Cross-chunk affine composition in f64 log-space. Inputs [K, BL].

    hst is the merged reverse scan at position 1 of each chunk (H_1);
    Dt = ia_0 * H_1 with ia_0 = exp(-BETA*log1p(r0)) from the raw input.
    """
    with np.errstate(divide="ignore"):
        l_gi = np.log(gi.astype(np.float64))
        l_dt = (-BETA * np.log1p(r0.astype(np.float64))
                + np.log(hst.astype(np.float64)))
        l_ve = np.log(vend.astype(np.float64))
    l_we = -BETA * sig.astype(np.float64)
    n = sig.shape[1]
    log_g = np.full(n, -np.inf)
    log_vstart = np.full(n, -np.inf)
    for k in range(K):
        log_g = np.logaddexp(log_g, np.logaddexp(l_gi[k], l_dt[k] + log_vstart))
        log_vstart = np.logaddexp(l_ve[k], l_we[k] + log_vstart)
    soft_mdd_log = log_g / BETA
    mdd = 1.0 - np.exp(-soft_mdd_log)
    return MDD_LAMBDA * np.maximum(mdd - THRESHOLD, 0.0)


def _run(path_returns, trace=False):
    nc = _build()
    xt = np.ascontiguousarray(path_returns.T)          # [B, T]
    r0 = path_returns[::C, :]                          # [K, B] chunk starts
    in_maps = [{"xt": xt[c * BL:(c + 1) * BL]} for c in range(NCORES)]
    res = run_bass_kernel_spmd(nc, in_maps, list(range(NCORES)), trace=trace)
    out = np.empty(B, np.float64)
    for c in range(NCORES):
        r = res.results[c]
        out[c * BL:(c + 1) * BL] = _combine(
            r["sig"].T, r["hst"].T, r["vend"].T, r["gi"].T,
            r0[:, c * BL:(c + 1) * BL])
    return out.astype(np.float32), res


def kernel(path_returns):
    out, _ = _run(path_returns)
    return out



# revision 3
# speedup vs baseline: 1.0369x; 1.0369x over previous
"""Trainium2 Bass kernel for SoftPathMDDLoss — transposed PE-bilinear design.

Math (beta-scaled): per 128-step subblock with strict suffix sums
zr'_t = sum_{s>t} ln(1+r_s), er = e^{B zr'}, emr = e^{-B zr'}:
    gi  = sum_t er_t * (L emr)_t          (in-block soft-drawdown sum)
    Ep  = (L emr)_127 = sum emr           (block-local V at block end)
    Em  = e^{-B sig} * sum er             (sum of prefix products)
    sig = sum_t b_t                       (exact f32 sum of fp16 b)
Cross-subblock composition runs on the host in f64 log space.

Device layout: time-within-subblock on partitions, (path, subblock) on the
free axis (host pre-transposes; tile j of 512 columns = path j's 512
subblocks). Four PE passes per element: zr' = U'.b (suffix cumsums; the
augmented ones-column makes er row 127 = e^{B sig}, DMA'd out for exact sig),
Sm = L.emr (prefix sums), and two rotating one-hot-stationary passes that
accumulate the per-subblock partition sums of erb (-> Em) and Z (-> gi) into
two long-lived PSUM banks, one row per subblock tile. ACT does Ln and the
two exps (PSUM-fed, bf16 out); DVE does the fused Z = er*(L emr) multiply
and the er f32->bf16 cast. Ep rides out as Z's row 127 with exact host-side
corrections for the augmented column. The whole quad chain is software-
pipelined: consume work is deferred 2 quads (Sm/stt) and 4 quads (rot-gi)
so PE never waits on ACT/DVE.
"""
import numpy as np

import concourse.bacc as bacc
import concourse.mybir as mybir
import concourse.tile as tile
from concourse.bass_utils import run_bass_kernel_spmd

T, B = 65536, 1024
NCORES = 8
PL = 128                  # paths per core (tile j <-> path j)
SB = 128                  # subblock length = partitions
Q = T // SB               # 512 subblocks per path
F = PL * Q                # 65536 free columns per core
BT = 8192                 # big-tile cols (Ln granularity)
QUAD = 2048               # exp span (4 psum banks)
ST = 512                  # subtile cols (one psum bank)
BETA = 40.0
THRESHOLD = 0.12
MDD_LAMBDA = 5.0

F32 = mybir.dt.float32
BF16 = mybir.dt.bfloat16
FP16 = mybir.dt.float16
AF = mybir.ActivationFunctionType
ALU = mybir.AluOpType

_built = {}


def _patch_act_tables():
    import concourse.hw_specs as hw_specs

    if getattr(bacc, "_act_tables_patched", False):
        return
    orig = hw_specs.get_activation_tables

    def patched(arch):
        t = orig(arch)
        pref = "natural_log_exp_and_others"
        if pref not in t:
            return t
        strip = {mybir.ActivationFunctionType.Exp,
                 mybir.ActivationFunctionType.Copy}
        return {k: (v if k == pref else (v - strip)) for k, v in t.items()}

    bacc.get_activation_tables = patched
    bacc._act_tables_patched = True


def build(f_cols=F):
    if f_cols in _built:
        return _built[f_cols]
    _patch_act_tables()
    nst = f_cols // ST            # subtiles (= rot group length, <=128)
    nbt = f_cols // BT if f_cols >= BT else 1
    bt = min(BT, f_cols)
    quads_per_bt = bt // QUAD
    assert nst <= 128

    nc = bacc.Bacc("TRN2", target_bir_lowering=False, debug=False)
    xt = nc.dram_tensor("xt", [SB, f_cols], F32, kind="ExternalInput").ap()
    cu = nc.dram_tensor("cu", [SB, SB], F32, kind="ExternalInput").ap()
    cl = nc.dram_tensor("cl", [SB, SB], F32, kind="ExternalInput").ap()
    cz = nc.dram_tensor("cz", [SB, 255], F32, kind="ExternalInput").ap()
    o_ser = nc.dram_tensor("ser", [128, ST], F32, kind="ExternalOutput").ap()
    o_gi = nc.dram_tensor("gi", [128, ST], F32, kind="ExternalOutput").ap()
    o_ep = nc.dram_tensor("ep", [128, ST], BF16, kind="ExternalOutput").ap()
    o_sg = nc.dram_tensor("sg", [max(f_cols // QUAD, 1), QUAD], F32,
                          kind="ExternalOutput").ap()

    with tile.TileContext(nc) as tc:
        with tc.tile_pool(name="io", bufs=2) as iop, \
             tc.tile_pool(name="wk", bufs=2) as wp, \
             tc.tile_pool(name="st", bufs=1) as sp, \
             tc.tile_pool(name="ps", bufs=1,
                          space=bacc.bass.MemorySpace.PSUM) as pp:
            # first input tile DMA issued before the constant loads
            xt0 = iop.tile([SB, bt], F32, tag="xt", name="xt")
            qtr0 = bt // 4
            for h in range(4):
                nc.sync.dma_start(
                    out=xt0[:, h * qtr0:(h + 1) * qtr0],
                    in_=xt[:, h * qtr0:(h + 1) * qtr0])
            # constants
            cuf = sp.tile([SB, SB], F32, tag="cuf", name="cuf")
            clf = sp.tile([SB, SB], F32, tag="clf", name="clf")
            czf = sp.tile([SB, 255], F32, tag="czf", name="czf")
            nc.sync.dma_start(out=cuf[:], in_=cu)
            nc.sync.dma_start(out=clf[:], in_=cl)
            nc.sync.dma_start(out=czf[:], in_=cz)
            uh = sp.tile([SB, SB], FP16, tag="uh", name="uh")
            lb = sp.tile([SB, SB], BF16, tag="lb", name="lb")
            zozb = sp.tile([SB, 255], BF16, tag="zozb", name="zozb")
            nc.vector.tensor_copy(uh[:], cuf[:])
            nc.vector.tensor_copy(lb[:], clf[:])
            nc.vector.tensor_copy(zozb[:], czf[:])

            # long-lived psum accumulators (2 banks)
            rser = pp.tile([128, ST], F32, tag="rser", name="rser")
            rgi = pp.tile([128, ST], F32, tag="rgi", name="rgi")

            ztss = {}

            def consume_a(prev):
                """rser + Sm/stt for quad qi_p (inputs ready)."""
                erf_p, erb_p, emr_p, qi_p = prev
                rser_mm = lambda s: nc.tensor.matmul(
                    rser[:], zozb[:, 127 - (qi_p * 4 + s):255 - (qi_p * 4 + s)],
                    erb_p[:, s * ST:(s + 1) * ST],
                    start=(qi_p * 4 + s == 0),
                    stop=(qi_p * 4 + s == nst - 1), skip_group_check=True)
                def do_sm(s):
                    smt = pp.tile([128, ST], F32, tag="sm", name="sm",
                                  bufs=2)
                    nc.tensor.matmul(smt[:], lb[:],
                                     emr_p[:, s * ST:(s + 1) * ST])
                    zt = wp.tile([128, ST], BF16, tag="zt", name="zt",
                                 bufs=16)
                    nc.vector.scalar_tensor_tensor(
                        zt[:], smt[:], 1.0, erf_p[:, s * ST:(s + 1) * ST],
                        ALU.bypass, ALU.mult)
                    j = qi_p * 4 + s
                    nc.sync.dma_start(out=o_ep[j:j + 1, :],
                                      in_=zt[127:128, :])
                    return zt
                zts = [do_sm(0), do_sm(1)]
                for s in range(4):
                    rser_mm(s)
                zts += [do_sm(2), do_sm(3)]
                ztss[qi_p] = zts

            def consume_b(qi_p):
                """rot-gi for a quad whose stts finished long ago."""
                zts = ztss.pop(qi_p)
                for s in range(4):
                    j = qi_p * 4 + s
                    nc.tensor.matmul(rgi[:], zozb[:, 127 - j:255 - j],
                                     zts[s][:],
                                     start=(j == 0), stop=(j == nst - 1),
                                     skip_group_check=True)

            nquads = f_cols // QUAD
            xt_tiles = {}

            def load_bt(ibt):
                xt_t = iop.tile([SB, bt], F32, tag="xt", name="xt")
                qtr = bt // 4
                for h in range(4):
                    nc.sync.dma_start(
                        out=xt_t[:, h * qtr:(h + 1) * qtr],
                        in_=xt[:, ibt * bt + h * qtr:ibt * bt + (h + 1) * qtr])
                xt_tiles[ibt] = xt_t

            def ln_quad(qi):
                """Ln for quad qi's 2048-col slice -> fp16 bt tile."""
                ibt = (qi * QUAD) // bt
                off = qi * QUAD - ibt * bt
                bt_sl = wp.tile([SB, QUAD], FP16, tag="bt", name="bt",
                                bufs=4)
                nc.scalar.activation(bt_sl[:], xt_tiles[ibt][:, off:off + QUAD],
                                     AF.Ln, bias=1.0, scale=1.0)
                return bt_sl

            xt_tiles[0] = xt0
            pend = {0: (ln_quad(0), 0)}
            hist = {}
            for qi in range(nquads):
                if qi % quads_per_bt == 0:
                    nxt_ibt = qi // quads_per_bt + 1
                    if nxt_ibt < nbt:
                        load_bt(nxt_ibt)
                if qi - 4 in ztss:
                    consume_b(qi - 4)
                if qi - 2 in hist:
                    consume_a(hist.pop(qi - 2))
                bt_sl, boff = pend.pop(qi)
                zqa = pp.tile([128, QUAD // 2], F32, tag="zqa", name="zqa")
                zqb = pp.tile([128, QUAD // 2], F32, tag="zqb", name="zqb")
                for s in range(2):
                    nc.tensor.matmul(zqa[:, s * ST:(s + 1) * ST], uh[:],
                                     bt_sl[:, boff + s * ST:boff + (s + 1) * ST])
                for s in range(2, 4):
                    nc.tensor.matmul(zqb[:, (s - 2) * ST:(s - 1) * ST], uh[:],
                                     bt_sl[:, boff + s * ST:boff + (s + 1) * ST])
                if qi + 1 < nquads:
                    pend[qi + 1] = (ln_quad(qi + 1), 0)
                erf = wp.tile([128, QUAD], F32, tag="erf", name="erf",
                              bufs=3)
                nc.scalar.activation(erf[:, :QUAD // 2], zqa[:], AF.Exp,
                                     bias=0.0, scale=BETA)
                nc.scalar.activation(erf[:, QUAD // 2:], zqb[:], AF.Exp,
                                     bias=0.0, scale=BETA)
                nc.sync.dma_start(out=o_sg[qi:qi + 1, :],
                                  in_=erf[127:128, :])
                erb = wp.tile([128, QUAD], BF16, tag="erb", name="erb",
                              bufs=3)
                nc.vector.tensor_copy(erb[:], erf[:])
                emr = wp.tile([128, QUAD], BF16, tag="emr", name="emr",
                              bufs=3)
                nc.scalar.activation(emr[:, :QUAD // 2], zqa[:], AF.Exp,
                                     bias=0.0, scale=-BETA)
                nc.scalar.activation(emr[:, QUAD // 2:], zqb[:], AF.Exp,
                                     bias=0.0, scale=-BETA)
                hist[qi] = (erf, erb, emr[:], qi)
            for qi in sorted(hist):
                consume_a(hist.pop(qi))
            for qi in sorted(ztss):
                consume_b(qi)

            eo = sp.tile([128, ST], F32, tag="eo", name="eo")
            go = sp.tile([128, ST], F32, tag="go", name="go")
            nc.vector.tensor_copy(eo[:], rser[:])
            nc.vector.tensor_copy(go[:], rgi[:])
            nc.sync.dma_start(out=o_ser, in_=eo[:])
            nc.sync.dma_start(out=o_gi, in_=go[:])

    nc.compile()
    _built[f_cols] = nc
    return nc


def make_consts():
    cu = np.tril(np.ones((SB, SB), np.float32), -1)      # [p, j]: p > j
    cu[:, 127] = 1.0                                      # row 127 -> sig
    cl = np.triu(np.ones((SB, SB), np.float32))          # [p, j]: p <= j
    cz = np.zeros((SB, 255), np.float32)
    cz[:, 127] = 1.0
    return cu, cl, cz


def prep_core(r, c, q=Q):
    """r [T', B] -> device layout [SB, F'] for core c (path-major cols)."""
    sub = r[:, c * PL:(c + 1) * PL]                      # [T', PL]
    a = sub.reshape(q, SB, PL)                            # (q, p, path)
    return np.ascontiguousarray(
        a.transpose(1, 2, 0).reshape(SB, PL * q), np.float32)


def _bf16(v):
    u = np.asarray(v, np.float32).view(np.uint32).astype(np.uint64)
    u = (u + 0x7FFF + ((u >> 16) & 1)) >> 16
    return (u.astype(np.uint32) << 16).view(np.float32).astype(np.float64)


def combine(er127, ser, gi, z127):
    """er127 = e^{B sig}; z127 = bf16(er127*Sm127); ser/gi rot sums [PL, Q']."""
    er127 = er127.astype(np.float64)
    SIG = np.log(er127)
    emsig = 1.0 / er127                                   # e^{-B sig}
    ep = emsig * z127.astype(np.float64) - emsig + 1.0    # true sum emr
    gi_t = gi.astype(np.float64) - z127.astype(np.float64) + ep
    ser_t = ser.astype(np.float64) - _bf16(er127) + 1.0
    with np.errstate(divide="ignore", invalid="ignore"):
        l_gi = np.log(gi_t)
        l_em = -SIG + np.log(ser_t)
        l_vend = np.log(ep)
    l_wend = -SIG
    nq = er127.shape[1]
    LG = np.full(er127.shape[0], -np.inf)
    LV = np.full(er127.shape[0], -np.inf)
    for qq in range(nq):
        LG = np.logaddexp(LG, np.logaddexp(l_gi[:, qq], l_em[:, qq] + LV))
        LV = np.logaddexp(l_vend[:, qq], l_wend[:, qq] + LV)
    soft_mdd_log = LG / BETA
    mdd = 1.0 - np.exp(-soft_mdd_log)
    return MDD_LAMBDA * np.maximum(mdd - THRESHOLD, 0.0)


def _run(path_returns, trace=False):
    nc = build()
    cu, cl, cz = make_consts()
    in_maps = [{"xt": prep_core(path_returns, c), "cu": cu, "cl": cl, "cz": cz}
               for c in range(NCORES)]
    res = run_bass_kernel_spmd(nc, in_maps, list(range(NCORES)), trace=trace)
    out = np.empty(B, np.float64)
    for c in range(NCORES):
        r = res.results[c]
        out[c * PL:(c + 1) * PL] = combine(
            np.asarray(r["sg"], np.float32).reshape(PL, Q),
            r["ser"].reshape(PL, Q), r["gi"].reshape(PL, Q),
            np.asarray(r["ep"], np.float32).reshape(PL, Q))
    return out.astype(np.float32), res


def kernel(path_returns):
    out, _ = _run(path_returns)
    return out
